# revision 1
# baseline (speedup 1.0000x reference)
"""Trainium2 Bass kernel for nn_FLinemodel_37185826849029.

Model (per batch b):
  Q = x@wq, K = x@wk, V = x@wv          [S,4]
  L = (Q K^T) @ W_at + b_at             [S,S]   <- rewritten as Q @ (K^T W_at)
  A = softmax(L, axis=-1)
  y = A @ V                             [S,4]
  p = softmax(y @ w_cls + b_cls)        [S,10]
  out = mean_s p                        [10]

Key algebraic rewrite: (Q K^T) W_at == Q (K^T W_at) since D=4, collapsing
~550 GFLOP to ~4 GFLOP.  Everything runs in a "transposed" layout (L^T
tiles [u,s]) so the softmax denominator comes free from the attend matmul
via a ones-column appended to V, and the classifier-stage sums come from a
unit column appended to w_cls.

Sequence axes use a grouped order: partition g holds rows [R*g, R*g+R)
(R = S/128), so every large DMA is contiguous per partition.  The s axis
(queries) is permutation-invariant through the final mean; the u axis uses
strided tiles {u == r (mod R)} so K/V/W_at/b_at all stay consistent.

Sharding: data-parallel over batch. 32 batches / 8 cores = 4 per core;
batches are packed into the PE array concurrently via tile_position.
"""

from contextlib import ExitStack

import numpy as np

import concourse.bacc as bacc
import concourse.mybir as mybir
import concourse.tile as tile
from concourse import masks

F32 = mybir.dt.float32
EXP = mybir.ActivationFunctionType.Exp
P = 128

B, S_FULL, H_FULL, D, C = 32, 2048, 256, 4, 10
N_CORES = 8
B_LOC = B // N_CORES


def build_nc(b_loc=B_LOC, s=S_FULL, h=H_FULL, reps=1):
    HC = h // P            # h chunks (2)
    R = s // P             # rows per partition-group; also # of u-tiles
    SC = s // 512          # 512-wide s chunks
    RC = R                 # W_at in one grouped DMA
    E = C + 1
    assert HC == 2 and s % 512 == 0 and R % 2 == 0 and R % RC == 0

    nc = bacc.Bacc("TRN2", debug=False, target_bir_lowering=False)

    xs_t = nc.dram_tensor("xs", [b_loc, s, h], F32, kind="ExternalInput")
    wq_t = nc.dram_tensor("wq", [h, D], F32, kind="ExternalInput")
    wk_t = nc.dram_tensor("wk", [h, D], F32, kind="ExternalInput")
    wv_t = nc.dram_tensor("wv", [h, D], F32, kind="ExternalInput")
    wat_t = nc.dram_tensor("w_at", [s, s], F32, kind="ExternalInput")
    bat_t = nc.dram_tensor("b_at", [s], F32, kind="ExternalInput")
    wcls_t = nc.dram_tensor("w_cls", [D, C], F32, kind="ExternalInput")
    bcls_t = nc.dram_tensor("b_cls", [C], F32, kind="ExternalInput")
    out_t = nc.dram_tensor("out", [b_loc, C], F32, kind="ExternalOutput")

    xs, wat = xs_t.ap(), wat_t.ap()

    with ExitStack() as ctx:
        tc = ctx.enter_context(tile.TileContext(nc))
        const = ctx.enter_context(tc.tile_pool(name="const", bufs=1))
        big = ctx.enter_context(tc.tile_pool(name="big", bufs=1))

        ident = const.tile([P, P], F32)
        masks.make_identity(nc, ident[:])
        ones_col = const.tile([P, 1], F32)
        nc.vector.memset(ones_col[:], 1.0)

        # w{q,kv}_sb[p, hc, d] = w[hc*P + p, d]; loaded flat (1 descriptor each)
        # and redistributed across partitions with tiny PE transposes.
        wq_sb = const.tile([P, HC, D], F32)
        wkv_sb = const.tile([P, HC, 2 * D], F32)
        wflat = const.tile([1, 3, h * D], F32)
        nc.sync.dma_start(wflat[:, 0], wq_t.ap().rearrange("h d -> (h d)")[None, :])
        nc.sync.dma_start(wflat[:, 1], wk_t.ap().rearrange("h d -> (h d)")[None, :])
        nc.sync.dma_start(wflat[:, 2], wv_t.ap().rearrange("h d -> (h d)")[None, :])

        # bat_sb[g, r] = b_at[R*g + r]  (grouped u order)
        bat_sb = const.tile([P, R], F32)
        nc.sync.dma_start(bat_sb[:], bat_t.ap().rearrange("(g r) -> g r", g=P))

        # classifier weights extended: rows (d0..d3, bias), cols (c0..c9, unit),
        # replicated at partition strips 32b for the row-tiled z matmuls.
        wce = const.tile([P, b_loc * E], F32)
        nc.vector.memset(wce[:], 0.0)
        for b in range(b_loc):
            nc.sync.dma_start(wce[32 * b : 32 * b + D, E * b : E * b + C], wcls_t.ap())
            nc.sync.dma_start(
                wce[32 * b + D : 32 * b + D + 1, E * b : E * b + C], bcls_t.ap()[None, :])
            # unit entry at (row D, col E*b + C) of strip b
            nc.gpsimd.affine_select(
                out=wce[32 * b : 32 * b + 32, :],
                in_=wce[32 * b : 32 * b + 32, :],
                pattern=[[1, b_loc * E]],
                compare_op=mybir.AluOpType.not_equal,
                fill=1.0,
                base=-(b_loc * E * D + E * b + C),
                channel_multiplier=b_loc * E,
            )

        qt_sb = big.tile([P, s], F32)               # rows 32b..: Q_b^T [4, s']
        k_pad = big.tile([P, R, P], F32)            # col 32b+d = K_b[R*g+r, d], else 0
        v_sb = big.tile([P, b_loc, R, D + 1], F32)  # V[R*g+r, d] + ones col
        m_sb = big.tile([P, s], F32)                # rows 32b..: M^T, col 128*rt+g <-> u=R*g+rt
        yt_sb = big.tile([P, s], F32)               # rows 32b..: [yhat^T; rowsum]

        nc.vector.memset(v_sb[:], 1.0)
        nc.vector.memset(k_pad[:], 0.0)

        with tc.tile_pool(name="ps_w", bufs=1, space="PSUM") as ps_w:
            wtp = ps_w.tile([P, 3 * HC * D], F32)
            wfv = wflat[:].rearrange("o w (hh hl d) -> o w hh d hl", hh=HC, d=D)
            for w in range(3):
                for hc in range(HC):
                    for d in range(D):
                        nc.tensor.transpose(
                            wtp[:, (w * HC + hc) * D + d : (w * HC + hc) * D + d + 1],
                            wfv[:, w, hc, d, :],
                            ident[0:1, 0:1],
                        )
            nc.vector.tensor_copy(
                wq_sb[:], wtp[:, : HC * D].rearrange("p (hh d) -> p hh d", d=D))
            nc.vector.tensor_copy(
                wkv_sb[:].rearrange("p hh (w d) -> p w hh d", d=D),
                wtp[:, HC * D :].rearrange("p (w hh d) -> p w hh d", hh=HC, d=D),
            )

        for _rep in range(reps):
            # ---- stage 1: x load (grouped), transpose, K/V/Q projections ----
            with tc.tile_pool(name="xload", bufs=2) as xload, \
                 tc.tile_pool(name="xtp", bufs=2) as xtp, \
                 tc.tile_pool(name="ps_stg", bufs=3, space="PSUM") as ps_stg, \
                 tc.tile_pool(name="ps_qt", bufs=4, space="PSUM") as ps_qt, \
                 tc.tile_pool(name="ps_kv", bufs=1, space="PSUM") as ps_kv:
                kv_full = ps_kv.tile([P, 512], F32)
                kv_ps = kv_full[:, : R * b_loc * 2 * D].rearrange(
                    "p (r b e) -> p r b e", r=R, b=b_loc)
                tp_tiles = [ps_stg.tile([P, 4, P], F32, tag="tp", name="tp") for _ in range(3)]
                qt_ps = [ps_qt.tile([P, 512], F32, tag="qt", name="qt") for _ in range(SC)]
                for scq in range(SC):
                    nc.vector.memset(qt_ps[scq][:], 0.0)
                for b in range(b_loc):
                    x_sb = xload.tile([P, R, h], F32, tag="x", name="x")
                    nc.sync.dma_start(x_sb[:], xs[b].rearrange("(g r) h -> g r h", g=P))
                    xT = xtp.tile([P, HC, s], F32, tag="xT", name="xT")
                    for j in range(R // 2):
                        tp = tp_tiles[j % 3]
                        for rr in range(2):
                            for hc in range(HC):
                                nc.tensor.transpose(
                                    tp[:, 2 * rr + hc, :],
                                    x_sb[:, 2 * j + rr, hc * P : (hc + 1) * P],
                                    ident[:],
                                )
                        dst = xT[:].rearrange(
                            "p hc (j rr f) -> p j rr hc f", rr=2, f=P)[:, j]
                        nc.vector.tensor_copy(
                            dst, tp[:].rearrange("p (rr hc) f -> p rr hc f", rr=2))
                    for r in range(R):
                        for hc in range(HC):
                            nc.tensor.matmul(
                                kv_ps[:, r, b, :],
                                xT[:, hc, r * P : (r + 1) * P],
                                wkv_sb[:, hc, :],
                                start=(hc == 0),
                                stop=(hc == HC - 1),
                            )
                    for scq in range(SC):
                        for hc in range(HC):
                            nc.tensor.matmul(
                                qt_ps[scq][32 * b : 32 * b + D, :],
                                wq_sb[:, hc, :],
                                xT[:, hc, 512 * scq : 512 * (scq + 1)],
                                start=(hc == 0),
                                stop=(hc == HC - 1),
                                skip_group_check=True,
                                tile_position=(0, 32 * b),
                            )
                nc.vector.tensor_copy(
                    k_pad[:].rearrange("p r (b e) -> p r b e", e=32)[:, :, 0:b_loc, 0:D],
                    kv_ps[:, :, :, 0:D],
                )
                nc.vector.tensor_copy(
                    v_sb[:, :, :, 0:D],
                    kv_ps[:, :, :, D : 2 * D].rearrange("p r b d -> p b r d"),
                )
                for scq in range(SC):
                    nc.vector.tensor_copy(
                        qt_sb[:, 512 * scq : 512 * (scq + 1)], qt_ps[scq][:]
                    )

            # ---- stage 2: M^T = K^T W_at, W_at streamed in grouped r-chunks ----
            with tc.tile_pool(name="wg", bufs=1) as wg_pool, \
                 tc.tile_pool(name="ps_m", bufs=1, space="PSUM") as ps_m:
                m_ps = ps_m.tile([P, s], F32)
                tpb = 512 // P  # u-tiles per psum bank
                for rc in range(R // RC):
                    wg = wg_pool.tile([P, RC, s], F32, tag="wg", name="wg")
                    nc.sync.dma_start(
                        wg[:],
                        wat.rearrange("(g r) u -> g r u", g=P)[:, rc * RC : (rc + 1) * RC],
                    )
                    for rr in range(RC):
                        r = rc * RC + rr
                        # rhs free order: (rt within bank, g) <-> u = R*g + rt
                        wview = wg[:, rr].rearrange("p (g rt) -> p rt g", rt=R)
                        for uc in range(SC):
                            nc.tensor.matmul(
                                m_ps[:, 512 * uc : 512 * (uc + 1)],
                                k_pad[:, r, :],
                                wview[:, uc * tpb : (uc + 1) * tpb],
                                start=(r == 0),
                                stop=(r == R - 1),
                            )
                nc.vector.tensor_copy(m_sb[:], m_ps[:])

            # ---- stage 3: attend, single pass, b_loc-wide exp groups ----
            with tc.tile_pool(name="esb", bufs=2) as e_pool, \
                 tc.tile_pool(name="ps_y", bufs=max(SC, 1), space="PSUM") as ps_y, \
                 tc.tile_pool(name="ps_l", bufs=1, space="PSUM") as ps_l:
                y_ps = [ps_y.tile([P, 512], F32, tag="y", name="y") for _ in range(SC)]
                for sc in range(SC):
                    nc.vector.memset(y_ps[sc][:], 0.0)
                le_tiles = [
                    (ps_l.tile([P, b_loc, 512], F32, tag="l", name="l"),
                     e_pool.tile([P, b_loc, 512], F32, tag="e", name="e"))
                    for _ in range(1)
                ]
                for rt in range(R):
                    for sc in range(SC):
                        l_ps, e_sb = le_tiles[(rt * SC + sc) % 1]
                        for i in range(b_loc):
                            nc.tensor.matmul(
                                l_ps[:, i, :],
                                m_sb[32 * i : 32 * i + D, rt * P : (rt + 1) * P],
                                qt_sb[32 * i : 32 * i + D, 512 * sc : 512 * (sc + 1)],
                                start=True,
                                stop=True,
                                tile_position=(32 * i, 0),
                            )
                        nc.scalar.activation(
                            e_sb[:], l_ps[:], EXP, bias=bat_sb[:, rt : rt + 1], scale=1.0
                        )
                        for i in range(b_loc):
                            nc.tensor.matmul(
                                y_ps[sc][32 * i : 32 * i + D + 1, :],
                                v_sb[:, i, rt, :],
                                e_sb[:, i, :],
                                start=(rt == 0),
                                stop=(rt == R - 1),
                                skip_group_check=True,
                                tile_position=(0, 32 * i),
                            )
                for sc in range(SC):
                    nc.vector.tensor_copy(
                        yt_sb[:, 512 * sc : 512 * (sc + 1)], y_ps[sc][:]
                    )

            # ---- epilogue: classifier + softmax + mean over s ----
            with tc.tile_pool(name="ep", bufs=2) as ep, \
                 tc.tile_pool(name="ps_z", bufs=2, space="PSUM") as ps_z, \
                 tc.tile_pool(name="ps_o", bufs=1, space="PSUM") as ps_o:
                out_ps = ps_o.tile([1, 512], F32)
                KR = next(kr for kr in (16, 8, 4, 2, 1)
                          if kr * b_loc * E <= 512 and R % kr == 0)
                for kh in range(R // KR):
                    z_full = ps_z.tile([P, 512], F32, tag="zf", name="zf")
                    z_ps = z_full[:, : KR * b_loc * E].rearrange(
                        "p (k i e) -> p k i e", k=KR, i=b_loc)
                    for kk in range(KR):
                        k = kh * KR + kk
                        nc.tensor.matmul(
                            z_ps[:, kk, :, :].rearrange("p i e -> p (i e)"),
                            yt_sb[:, k * P : (k + 1) * P],
                            wce[:],
                            start=True,
                            stop=True,
                        )
                    r_sb = ep.tile([P, KR * b_loc], F32, tag="r", name="r")
                    nc.vector.reciprocal(r_sb[:], z_ps[:, :, :, C])
                    zz = ep.tile([P, KR, b_loc, C], F32, tag="zz", name="zz")
                    nc.vector.tensor_tensor(
                        zz[:],
                        z_ps[:, :, :, 0:C],
                        r_sb[:].rearrange("p (k i) -> p k i", k=KR)
                            .unsqueeze(-1).broadcast_to([P, KR, b_loc, C]),
                        mybir.AluOpType.mult,
                    )
                    ez = ep.tile([P, KR, b_loc, C], F32, tag="ez", name="ez")
                    nc.scalar.activation(ez[:], zz[:], EXP)
                    sz = ep.tile([P, KR * b_loc], F32, tag="sz", name="sz")
                    nc.vector.tensor_reduce(
                        sz[:], ez[:], axis=mybir.AxisListType.X, op=mybir.AluOpType.add
                    )
                    rz = ep.tile([P, KR * b_loc], F32, tag="rz", name="rz")
                    nc.vector.reciprocal(rz[:], sz[:])
                    pz = ep.tile([P, KR, b_loc, C], F32, tag="pz", name="pz")
                    nc.vector.tensor_tensor(
                        pz[:],
                        ez[:],
                        rz[:].rearrange("p (k i) -> p k i", k=KR)
                            .unsqueeze(-1).broadcast_to([P, KR, b_loc, C]),
                        mybir.AluOpType.mult,
                    )
                    pc_sb = ep.tile([P, b_loc, C], F32, tag="pc", name="pc")
                    nc.vector.tensor_reduce(
                        pc_sb[:],
                        pz[:].rearrange("p k i c -> p i c k"),
                        axis=mybir.AxisListType.X,
                        op=mybir.AluOpType.add,
                    )
                    nc.tensor.matmul(
                        out_ps[:, : b_loc * C],
                        ones_col[:],
                        pc_sb[:].rearrange("p i c -> p (i c)"),
                        start=(kh == 0),
                        stop=(kh == R // KR - 1),
                    )
                out_sb = ep.tile([1, b_loc * C], F32, tag="o", name="o")
                nc.scalar.mul(out_sb[:], out_ps[:, : b_loc * C], 1.0 / s)
                nc.sync.dma_start(out_t.ap().rearrange("b c -> (b c)")[None, :], out_sb[:])

    nc.finalize()
    return nc


_NC_CACHE = {}


def _get_nc(key=(B_LOC, S_FULL, H_FULL), reps=1):
    if (key, reps) not in _NC_CACHE:
        _NC_CACHE[(key, reps)] = build_nc(*key, reps=reps)
    return _NC_CACHE[(key, reps)]


def kernel(x, wq, wk, wv, w_at, b_at, w_cls, b_cls):
    from concourse.bass_utils import run_bass_kernel_spmd

    x = np.ascontiguousarray(np.asarray(x, dtype=np.float32))
    nc = _get_nc()
    shared = {
        "wq": np.asarray(wq, np.float32),
        "wk": np.asarray(wk, np.float32),
        "wv": np.asarray(wv, np.float32),
        "w_at": np.ascontiguousarray(np.asarray(w_at, np.float32)),
        "b_at": np.asarray(b_at, np.float32),
        "w_cls": np.asarray(w_cls, np.float32),
        "b_cls": np.asarray(b_cls, np.float32),
    }
    in_maps = [
        {"xs": x[c * B_LOC : (c + 1) * B_LOC], **shared} for c in range(N_CORES)
    ]
    last_err = None
    for _attempt in range(3):
        try:
            res = run_bass_kernel_spmd(nc, in_maps, list(range(N_CORES))).results
            return np.concatenate([res[c]["out"] for c in range(N_CORES)], axis=0)
        except Exception as e:  # transient NRT/axon execution failures
            last_err = e
    raise last_err

